# revision 1
# baseline (speedup 1.0000x reference)
"""Trainium2 Bass kernel: causal multi-head attention block (B=2,S=2048,H=2048,NH=16,HD=128).

Sharding: 8 cores = DP over batch (2) x TP over head-groups (4 groups of 4 heads).
Each core computes q/k/v projections for its 4 heads, RoPE, causal softmax
attention, and a partial output projection; the host sums the 4 partials per
batch and adds bo.

Device layouts (all chosen so every matmul streams natural contiguous-free
tiles; the host pre-transposes x and the weights and casts matmul operands to
bf16 -- accumulation stays fp32 in PSUM):
  xT   [H=2048(f), S=2048(s)]   = x[b].T                       bf16
  wqT  [2048(f), 512(d)]        = (Wq[rows]/sqrt(HD)).T        bf16
  wkT  [2048(f), 512(d)]        = Wk[rows].T                   bf16
  wvT  [2048(f), 512(d)]        = Wv[rows].T                   bf16
  woT  [512(d), 2048(o)]        = Wo[:, rows].T                bf16
Attention runs with transposed score tiles ST[k,q] so the P@V matmul needs no
on-chip transposes; row sums (softmax denominators) come from a ones-vector
matmul on the PE, and 1/denom is broadcast across partitions with a K=1 matmul.
"""

import math
import os
import sys

import numpy as np

for _p in ("/opt/trn_rl_repo",):
    if _p not in sys.path and os.path.isdir(_p):
        sys.path.insert(0, _p)

import ml_dtypes

import concourse.bass as bass
import concourse.mybir as mybir
import concourse.tile as tile
from concourse import bacc

B, S, H, NH, HD = 2, 2048, 2048, 16, 128
NCORES = 8
HG = 4            # head-groups (TP degree)
HPG = NH // HG    # heads per group = 4
DLOC = HPG * HD   # local d width = 512
FT = H // 128     # 16 f-tiles
SJ = S // 512     # 4 s/q tiles of 512
KT128 = S // 128  # 16 k-tiles of 128
NEG = -1e30

F32 = mybir.dt.float32
F16 = mybir.dt.float16
BF16 = mybir.dt.bfloat16
NPBF16 = ml_dtypes.bfloat16


def build_program(mode: str) -> bass.Bass:
    """mode in {'causal', 'full', 'bias'}"""
    # Collapse Tile's DMA-completion sem round-robin to one lane per DGE class.
    # SP-issued HWDGE DMAs retire FIFO, so a single counting sem stays sound,
    # and every consumer's DMA waits merge into one sem -- without this, waits
    # on 3+ distinct DMAHW*/DMASW* sems overflow the ISA sync-wait budget on
    # DMA instructions ("Too many sync wait commands" in walrus codegen).
    import concourse.tile_sem_assignment as tsa

    tsa.NUM_HWDGE_SEMS = 1
    tsa.NUM_SWDGE_GLOBAL_SEMS = 1
    nc = bacc.Bacc()
    xT = nc.dram_tensor("xT", [H, S], BF16, kind="ExternalInput")
    wqT = nc.dram_tensor("wqT", [H, DLOC], BF16, kind="ExternalInput")
    wkT = nc.dram_tensor("wkT", [H, DLOC], BF16, kind="ExternalInput")
    wvT = nc.dram_tensor("wvT", [H, DLOC], BF16, kind="ExternalInput")
    woT = nc.dram_tensor("woT", [DLOC, H], BF16, kind="ExternalInput")
    bqT = nc.dram_tensor("bqT", [128, HPG], F32, kind="ExternalInput")
    bkT = nc.dram_tensor("bkT", [128, HPG], F32, kind="ExternalInput")
    bv = nc.dram_tensor("bv", [128, DLOC], F32, kind="ExternalInput")
    cosT = nc.dram_tensor("cosT", [HD, S], F32, kind="ExternalInput")
    sinT = nc.dram_tensor("sinT", [HD, S], F32, kind="ExternalInput")
    rmat = nc.dram_tensor("rmat", [HD, HD], BF16, kind="ExternalInput")
    if mode == "causal":
        dbias = nc.dram_tensor("dbias", [4, 128, 512], F32, kind="ExternalInput")
    elif mode == "bias":
        fbias = nc.dram_tensor("fbias", [S, S], F32, kind="ExternalInput")
    y = nc.dram_tensor("y", [S, H], F32, kind="ExternalOutput")

    with tile.TileContext(nc) as tc:
        with (
            tc.tile_pool(name="qt", bufs=HPG * SJ) as qt_pool,
            tc.tile_pool(name="kt", bufs=HPG * SJ) as kt_pool,
            tc.tile_pool(name="vt", bufs=KT128) as vt_pool,
            tc.tile_pool(name="consts", bufs=1) as consts,
        ):
            QT = {}  # (h, sj) -> [128, 512] bf16 tile, RoPE'd q^T (pre-scaled)
            KT = {}  # (h, sj) -> [128, 512] bf16 tile, RoPE'd k^T
            VT = {}  # ssub -> [128(s), 512(d)] bf16 tile, v + bias

            # constants that DMA later phases' data are declared here but loaded
            # just-in-time to keep the single DMA FIFO's head free for phase 1
            ones_sb = consts.tile([128, 1], BF16, tag="ones")
            nc.gpsimd.memset(ones_sb[:], 1.0)
            onesr_sb = consts.tile([1, 128], F16, tag="onesr")
            nc.gpsimd.memset(onesr_sb[:], 1.0)
            bv_sb = consts.tile([128, DLOC], F32, tag="bv")
            wo_sb = consts.tile([128, HPG, H], BF16, tag="wo")
            db_sb = None
            if mode == "causal":
                db_sb = consts.tile([128, 4, 512], F32, tag="db")

            # ================= Phase 1: Q/K projections + RoPE =================
            with (
                tc.tile_pool(name="ps12", bufs=8, space="PSUM") as psum,
                tc.tile_pool(name="wqk", bufs=1) as wqk_pool,
                tc.tile_pool(name="csn", bufs=1) as csn_pool,
                tc.tile_pool(name="xin", bufs=2) as xin_pool,
                tc.tile_pool(name="rtmp", bufs=3) as rtmp_pool,
            ):
                # weight loads first: the first matmul waits on these.
                # wq and the first x chunk are split per f-tile and interleaved
                # so the first matmuls start ~1 us after the first two pieces.
                wq_sb = wqk_pool.tile([128, FT, DLOC], BF16, tag="wq")
                wk_sb = wqk_pool.tile([128, FT, DLOC], BF16, tag="wk")
                cos_sb = csn_pool.tile([HD, S], F32, tag="cos")
                sin_sb = csn_pool.tile([HD, S], F32, tag="sin")
                rmat_sb = consts.tile([HD, HD], BF16, tag="rmat")
                bq_sb = consts.tile([128, HPG], F32, tag="bq")
                bk_sb = consts.tile([128, HPG], F32, tag="bk")

                nc.sync.dma_start(
                    wq_sb[:], wqT.rearrange("(ft p) d -> p ft d", p=128)
                )
                nc.sync.dma_start(
                    wk_sb[:], wkT.rearrange("(ft p) d -> p ft d", p=128)
                )
                for sj in range(SJ):
                    # one 2 MiB chunk DMA per s-tile: [f-tile, f-part, s] cache
                    xt = xin_pool.tile([128, FT, 512], BF16, tag="xt", name="xt")
                    nc.sync.dma_start(
                        xt[:],
                        xT[:, sj * 512 : (sj + 1) * 512].rearrange(
                            "(ft p) s -> p ft s", p=128
                        ),
                    )
                    if sj == 0:
                        # behind wq+wk+xt0 on the DMA FIFO; Q matmuls hide them
                        nc.sync.dma_start(rmat_sb[:], rmat[:])
                        nc.sync.dma_start(bq_sb[:], bqT[:])
                        nc.sync.dma_start(bk_sb[:], bkT[:])
                        nc.sync.dma_start(cos_sb[:], cosT[:])
                        nc.sync.dma_start(sin_sb[:], sinT[:])
                    qp = [psum.tile([128, 512], F32, tag="ps", name="ps") for _ in range(HPG)]
                    kp = [psum.tile([128, 512], F32, tag="ps", name="ps") for _ in range(HPG)]
                    for ft in range(FT):
                        for h in range(HPG):
                            nc.tensor.matmul(
                                qp[h][:],
                                wq_sb[:, ft, h * 128 : (h + 1) * 128],
                                xt[:, ft, :],
                                start=(ft == 0),
                                stop=(ft == FT - 1),
                            )
                    for ft in range(FT):
                        for h in range(HPG):
                            nc.tensor.matmul(
                                kp[h][:],
                                wk_sb[:, ft, h * 128 : (h + 1) * 128],
                                xt[:, ft, :],
                                start=(ft == 0),
                                stop=(ft == FT - 1),
                            )
                    css = cos_sb[:, sj * 512 : (sj + 1) * 512]
                    sss = sin_sb[:, sj * 512 : (sj + 1) * 512]
                    for h in range(HPG):
                        for which, ps, bias_sb, store in (
                            ("q", qp[h], bq_sb, QT),
                            ("k", kp[h], bk_sb, KT),
                        ):
                            pool = qt_pool if which == "q" else kt_pool
                            t = pool.tile([128, 512], BF16, tag="t", name="qkt")
                            # t = bf16(psum + per-partition bias); frees the bank
                            nc.scalar.activation(
                                t[:],
                                ps[:],
                                mybir.ActivationFunctionType.Identity,
                                bias=bias_sb[:, h : h + 1],
                            )
                            # rot = R @ t  (RoPE rotate_half as a permutation matmul)
                            rp = psum.tile([128, 512], F32, tag="ps", name="ps")
                            nc.tensor.matmul(
                                rp[:], rmat_sb[:], t[:], start=True, stop=True
                            )
                            tmp = rtmp_pool.tile([128, 512], BF16, tag="tmp", name="tmp")
                            nc.vector.tensor_mul(tmp[:], rp[:], sss)
                            nc.vector.tensor_mul(t[:], t[:], css)
                            nc.vector.tensor_add(t[:], t[:], tmp[:])
                            store[(h, sj)] = t

                # ================= Phase 2: V projection =================
                with (
                    tc.tile_pool(name="wv", bufs=1) as wv_pool,
                    tc.tile_pool(name="xv", bufs=3) as xv_pool,
                ):
                    wv_sb = wv_pool.tile([128, FT, DLOC], BF16, tag="wv")
                    nc.sync.dma_start(
                        wv_sb[:], wvT.rearrange("(ft p) d -> p ft d", p=128)
                    )
                    nc.sync.dma_start(bv_sb[:], bv[:])
                    # phase-3 constants ride behind phase-2's weights on the FIFO
                    nc.sync.dma_start(
                        wo_sb[:], woT.rearrange("(dt p) o -> p dt o", p=128)
                    )
                    if mode == "causal":
                        nc.sync.dma_start(
                            db_sb[:], dbias.rearrange("a p t -> p a t")
                        )
                    for ss in range(KT128):
                        xv = xv_pool.tile([128, FT, 128], BF16, tag="xv", name="xv")
                        nc.sync.dma_start(
                            xv[:],
                            xT[:, ss * 128 : (ss + 1) * 128].rearrange(
                                "(ft p) s -> p ft s", p=128
                            ),
                        )
                        vp = psum.tile([128, 512], F32, tag="ps", name="ps")
                        for ft in range(FT):
                            nc.tensor.matmul(
                                vp[:], xv[:, ft, :], wv_sb[:, ft, :],
                                start=(ft == 0), stop=(ft == FT - 1),
                            )
                        v = vt_pool.tile([128, DLOC], BF16, tag="v", name="v")
                        nc.vector.tensor_add(v[:], vp[:], bv_sb[:])
                        VT[ss] = v

            # ============ Phase 3: attention + output projection ============
            with (
                tc.tile_pool(name="pst", bufs=2, space="PSUM") as psum_st,
                tc.tile_pool(name="ppv", bufs=2, space="PSUM") as psum_pv,
                tc.tile_pool(name="pdn", bufs=2, space="PSUM") as psum_dn,
                tc.tile_pool(name="pyp", bufs=2, space="PSUM") as psum_yp,
                tc.tile_pool(name="ex", bufs=6) as exp_pool,
                tc.tile_pool(name="ot", bufs=2 * HPG) as ot_pool,
                tc.tile_pool(name="rc", bufs=3) as rc_pool,
                tc.tile_pool(name="ysb", bufs=4) as y_pool,
                tc.tile_pool(name="fb", bufs=3) as fb_pool,
            ):
                for qj in range(SJ):
                    OT = {}
                    PV = {}
                    RCH = {}

                    def _normalize(i):
                        # deferred one head behind so the PE's rcb matmul never
                        # waits on the DVE reciprocal chain
                        rcb_ps = psum_st.tile([128, 512], F32, tag="st", name="rcb_ps")
                        nc.tensor.matmul(
                            rcb_ps[:], onesr_sb[:], RCH[i][:], start=True, stop=True
                        )
                        rcb = rc_pool.tile([128, 512], F16, tag="rcb", name="rcb")
                        nc.vector.tensor_copy(rcb[:], rcb_ps[:])
                        ot = ot_pool.tile([128, 512], BF16, tag="ot", name="ot")
                        nc.vector.tensor_mul(ot[:], PV[i][:], rcb[:])
                        OT[i] = ot

                    for h in range(HPG):
                        kmax = 4 * qj + 4 if mode == "causal" else KT128
                        pv = psum_pv.tile([128, 512], F32, tag="pv", name="pv")
                        dn = psum_dn.tile([1, 512], F32, tag="dn", name="dn")
                        for kj in range(kmax):
                            # columns of this q-tile that the k-tile can see at
                            # all (causal): the diagonal k-tile only reaches
                            # q >= its own first row.
                            a = kj - 4 * qj
                            off = 128 * a if (mode == "causal" and a > 0) else 0
                            n = 512 - off
                            st = psum_st.tile([128, 512], F32, tag="st", name="st")
                            nc.tensor.matmul(
                                st[:, off:],
                                KT[(h, kj // 4)][:, (kj % 4) * 128 : (kj % 4 + 1) * 128],
                                QT[(h, qj)][:, off:],
                                start=True,
                                stop=True,
                            )
                            if mode == "causal" and a >= 0:
                                # only the 128-wide band straddling the diagonal
                                # is partially masked
                                nc.vector.tensor_add(
                                    st[:, off : off + 128],
                                    st[:, off : off + 128],
                                    db_sb[:, a, off : off + 128],
                                )
                            elif mode == "bias":
                                fb = fb_pool.tile([128, 512], F32, tag="fb", name="fb")
                                nc.sync.dma_start(
                                    fb[:],
                                    fbias[
                                        kj * 128 : (kj + 1) * 128,
                                        qj * 512 : (qj + 1) * 512,
                                    ],
                                )
                                nc.vector.tensor_add(st[:], st[:], fb[:])
                            e = exp_pool.tile([128, 512], BF16, tag="e", name="e")
                            nc.scalar.activation(
                                e[:, off:], st[:, off:],
                                mybir.ActivationFunctionType.Exp,
                            )
                            nc.tensor.matmul(
                                pv[:, off:],
                                VT[kj][:, h * 128 : (h + 1) * 128],
                                e[:, off:],
                                start=(kj == 0),
                                stop=(kj == kmax - 1),
                            )
                            nc.tensor.matmul(
                                dn[:, off:],
                                ones_sb[:],
                                e[:, off:],
                                start=(kj == 0),
                                stop=(kj == kmax - 1),
                            )
                        rcf = rc_pool.tile([1, 512], F32, tag="rcf", name="rcf")
                        nc.vector.reciprocal_approx_fast(rcf[:], dn[:])
                        rch = rc_pool.tile([1, 512], F16, tag="rch", name="rch")
                        nc.vector.tensor_copy(rch[:], rcf[:])
                        PV[h] = pv
                        RCH[h] = rch
                        if h > 0:
                            _normalize(h - 1)
                        if h == HPG - 1:
                            _normalize(h)
                    # output projection for this q-tile of 512 rows
                    for ss in range(4):
                        for oj in range(4):
                            yp = psum_yp.tile([128, 512], F32, tag="yp", name="yp")
                            for dt in range(HPG):
                                nc.tensor.matmul(
                                    yp[:],
                                    OT[dt][:, ss * 128 : (ss + 1) * 128],
                                    wo_sb[:, dt, oj * 512 : (oj + 1) * 512],
                                    start=(dt == 0),
                                    stop=(dt == HPG - 1),
                                )
                            ysb = y_pool.tile([128, 512], F32, tag="y", name="y")
                            nc.vector.tensor_copy(ysb[:], yp[:])
                            r0 = qj * 512 + ss * 128
                            nc.sync.dma_start(
                                y[r0 : r0 + 128, oj * 512 : (oj + 1) * 512], ysb[:]
                            )
    nc.compile()
    return nc


_PROGRAM_CACHE = {}


def _get_program(mode):
    if mode not in _PROGRAM_CACHE:
        _PROGRAM_CACHE[mode] = build_program(mode)
    return _PROGRAM_CACHE[mode]


def _detect_mode(attn_mask):
    m = np.asarray(attn_mask).reshape(S, S)
    if (m == np.tril(np.ones((S, S), m.dtype))).all():
        return "causal"
    if (m != 0).all():
        return "full"
    return "bias"


def _rot_matrix():
    # rot(q)[d'] = -q[d'+1] (d' even), +q[d'-1] (d' odd);  rotT = R^T @ qT with
    # lhsT[d, d'] convention of nc.tensor.matmul.
    r = np.zeros((HD, HD), np.float32)
    for dp in range(HD):
        if dp % 2 == 0:
            r[dp + 1, dp] = -1.0
        else:
            r[dp - 1, dp] = 1.0
    return r


def _diag_bias():
    a = np.arange(4)[:, None, None]
    p = np.arange(128)[None, :, None]
    t = np.arange(512)[None, None, :]
    return np.where(128 * a + p <= t, 0.0, NEG).astype(np.float32)


def _bf16(a):
    return np.ascontiguousarray(a).astype(NPBF16)


def kernel(**inputs) -> np.ndarray:
    from concourse.bass_utils import run_bass_kernel_spmd

    x = np.asarray(inputs["x"], np.float32)
    fcos = np.asarray(inputs["fcos"], np.float32)
    fsin = np.asarray(inputs["fsin"], np.float32)
    Wq, bq = np.asarray(inputs["Wq"], np.float32), np.asarray(inputs["bq"], np.float32)
    Wk, bk = np.asarray(inputs["Wk"], np.float32), np.asarray(inputs["bk"], np.float32)
    Wv, bv = np.asarray(inputs["Wv"], np.float32), np.asarray(inputs["bv"], np.float32)
    Wo, bo = np.asarray(inputs["Wo"], np.float32), np.asarray(inputs["bo"], np.float32)
    attn_mask = inputs["attn_mask"]

    mode = _detect_mode(attn_mask)
    nc = _get_program(mode)

    sc = 1.0 / math.sqrt(HD)
    shared = {
        "cosT": np.ascontiguousarray(fcos.T),
        "sinT": np.ascontiguousarray(fsin.T),
        "rmat": _rot_matrix().astype(NPBF16),
    }
    if mode == "causal":
        shared["dbias"] = _diag_bias()
    elif mode == "bias":
        m = np.asarray(attn_mask).reshape(S, S)
        shared["fbias"] = np.ascontiguousarray(
            np.where(m.T == 0, NEG, 0.0).astype(np.float32)
        )

    in_maps = []
    for c in range(NCORES):
        b, hg = divmod(c, HG)
        rows = slice(DLOC * hg, DLOC * (hg + 1))
        in_maps.append(
            {
                "xT": _bf16(x[b].T),
                "wqT": _bf16((Wq[rows] * sc).T),
                "wkT": _bf16(Wk[rows].T),
                "wvT": _bf16(Wv[rows].T),
                "woT": _bf16(Wo[:, rows].T),
                "bqT": np.ascontiguousarray((bq[rows] * sc).reshape(HPG, 128).T),
                "bkT": np.ascontiguousarray(bk[rows].reshape(HPG, 128).T),
                "bv": np.ascontiguousarray(
                    np.broadcast_to(bv[rows].reshape(1, DLOC), (128, DLOC))
                ).astype(np.float32),
                **shared,
            }
        )

    trace = bool(int(os.environ.get("KERNEL_TRACE", "0")))
    res = run_bass_kernel_spmd(nc, in_maps, list(range(NCORES)), trace=trace)
    if trace and res.exec_time_ns is not None:
        print(f"HW exec time: {res.exec_time_ns} ns")
        globals()["LAST_EXEC_NS"] = res.exec_time_ns
        globals()["LAST_RESULTS"] = res

    out = np.zeros((B, S, H), np.float32)
    for c in range(NCORES):
        out[c // HG] += res.results[c]["y"]
    out += bo
    return out



# revision 2
# speedup vs baseline: 1.1256x; 1.1256x over previous
"""Trainium2 Bass kernel: causal multi-head attention block (B=2,S=2048,H=2048,NH=16,HD=128).

Sharding: 8 cores = DP over batch (2) x TP over head-groups (4 groups of 4 heads).
Each core computes q/k/v projections for its 4 heads, RoPE, causal softmax
attention, and a partial output projection; the host sums the 4 partials per
batch and adds bo.

Structure (v2, pipelined):
  Phase A (per 512-column s-chunk sj): Q, K, V projection matmuls back-to-back
  on the PE (x is loaded once and shared by all three), with Q/K evacuation
  (bias + RoPE) on ACT/DVE overlapped under the V matmuls. Startup DMAs are
  split per f-tile so the first matmul starts ~1us in.
  Phase B (per 512-row q-chunk qj, per head): scores are computed transposed
  ST[k,q] two k-tiles at a time into a 2-bank PSUM tile, masked via a
  precomputed additive -1e30 tile, exp'd in one [128,1024] ACTIVATE, and fed
  to the PV accumulation. Softmax denominators come from a DVE-accumulated
  e-sum followed by a single ones-vector matmul per (head, qj) instead of a
  per-k-tile PE matmul. The output projection of the previous qj is issued
  one matmul at a time between attention pairs to keep the PE dense (and the
  HAM clock un-throttled). y partials are written bf16; the host sums in f32.
"""

import math
import os
import sys

import numpy as np

for _p in ("/opt/trn_rl_repo",):
    if _p not in sys.path and os.path.isdir(_p):
        sys.path.insert(0, _p)

import ml_dtypes

import concourse.bass as bass
import concourse.mybir as mybir
import concourse.tile as tile
from concourse import bacc

B, S, H, NH, HD = 2, 2048, 2048, 16, 128
NCORES = 8
HG = 4            # head-groups (TP degree)
HPG = NH // HG    # heads per group = 4
DLOC = HPG * HD   # local d width = 512
FT = H // 128     # 16 f-tiles
SJ = S // 512     # 4 s/q tiles of 512
KT128 = S // 128  # 16 k-tiles of 128
NEG = -1e30

F32 = mybir.dt.float32
F16 = mybir.dt.float16
BF16 = mybir.dt.bfloat16
NPBF16 = ml_dtypes.bfloat16


def build_program(mode: str) -> bass.Bass:
    """mode in {'causal', 'full', 'bias'}"""
    # Collapse Tile's DMA-completion sem round-robin to one lane per DGE class
    # (see baseline notes: avoids ISA sync-wait budget overflow).
    import concourse.tile_sem_assignment as tsa

    tsa.NUM_HWDGE_SEMS = 1
    tsa.NUM_SWDGE_GLOBAL_SEMS = 1
    nc = bacc.Bacc()
    xT = nc.dram_tensor("xT", [H, S], BF16, kind="ExternalInput")
    wqT = nc.dram_tensor("wqT", [H, DLOC], BF16, kind="ExternalInput")
    wkT = nc.dram_tensor("wkT", [H, DLOC], BF16, kind="ExternalInput")
    wvT = nc.dram_tensor("wvT", [H, DLOC], BF16, kind="ExternalInput")
    woT = nc.dram_tensor("woT", [DLOC, H], BF16, kind="ExternalInput")
    bqT = nc.dram_tensor("bqT", [128, HPG], F32, kind="ExternalInput")
    bkT = nc.dram_tensor("bkT", [128, HPG], F32, kind="ExternalInput")
    bv = nc.dram_tensor("bv", [128, DLOC], F32, kind="ExternalInput")
    cosT = nc.dram_tensor("cosT", [HD, S], F32, kind="ExternalInput")
    sinT = nc.dram_tensor("sinT", [HD, S], F32, kind="ExternalInput")
    rmat = nc.dram_tensor("rmat", [HD, HD], BF16, kind="ExternalInput")
    if mode == "causal":
        dmask = nc.dram_tensor("dmask", [2, 128, 1024], F32, kind="ExternalInput")
    elif mode == "bias":
        fbias = nc.dram_tensor("fbias", [S, S], F32, kind="ExternalInput")
    y = nc.dram_tensor("y", [S, H], BF16, kind="ExternalOutput")

    with tile.TileContext(nc) as tc:
        with (
            tc.tile_pool(name="qt", bufs=HPG * SJ) as qt_pool,
            tc.tile_pool(name="kt", bufs=HPG * SJ) as kt_pool,
            tc.tile_pool(name="vt", bufs=KT128) as vt_pool,
            tc.tile_pool(name="consts", bufs=1) as consts,
        ):
            QT = {}  # (h, sj) -> [128, 512] bf16 tile, RoPE'd q^T (pre-scaled)
            KT = {}  # (h, sj) -> [128, 512] bf16 tile, RoPE'd k^T
            VT = {}  # ssub -> [128(s), 512(d)] bf16 tile, v + bias

            ones_sb = consts.tile([128, 1], F16, tag="ones")
            nc.gpsimd.memset(ones_sb[:], 1.0)
            onesr_sb = consts.tile([1, 128], F16, tag="onesr")
            nc.gpsimd.memset(onesr_sb[:], 1.0)
            bv_sb = consts.tile([128, DLOC], F32, tag="bv")
            wo_sb = consts.tile([128, HPG, H], BF16, tag="wo")
            dm_sb = None
            if mode == "causal":
                dm_sb = consts.tile([128, 2, 1024], F32, tag="dm")

            # ============ Phase A: fused Q/K/V projections + RoPE ============
            with (
                tc.tile_pool(name="pa", bufs=8, space="PSUM") as pa,
                tc.tile_pool(name="wqk", bufs=1) as wqk_pool,
                tc.tile_pool(name="csn", bufs=1) as csn_pool,
                tc.tile_pool(name="xin", bufs=2) as xin_pool,
                tc.tile_pool(name="rtmp", bufs=3) as rtmp_pool,
            ):
                wq_sb = wqk_pool.tile([128, FT, DLOC], BF16, tag="wq")
                wk_sb = wqk_pool.tile([128, FT, DLOC], BF16, tag="wk")
                wv_sb = wqk_pool.tile([128, FT, DLOC], BF16, tag="wv")
                cos_sb = csn_pool.tile([HD, S], F32, tag="cos")
                sin_sb = csn_pool.tile([HD, S], F32, tag="sin")
                rmat_sb = consts.tile([HD, HD], BF16, tag="rmat")
                bq_sb = consts.tile([128, HPG], F32, tag="bq")
                bk_sb = consts.tile([128, HPG], F32, tag="bk")

                xts = []
                for sj in range(SJ):
                    xts.append(xin_pool.tile([128, FT, 512], BF16, tag="xt", name="xt"))

                def evac_qk(which, h, ps, bias_sb, store, sj):
                    pool = qt_pool if which == "q" else kt_pool
                    t = pool.tile([128, 512], BF16, tag="t", name="qkt")
                    nc.scalar.activation(
                        t[:], ps[:],
                        mybir.ActivationFunctionType.Identity,
                        bias=bias_sb[:, h : h + 1],
                    )
                    rp = pa.tile([128, 512], F32, tag="ps", name="rp")
                    nc.tensor.matmul(rp[:], rmat_sb[:], t[:], start=True, stop=True)
                    tmp = rtmp_pool.tile([128, 512], BF16, tag="tmp", name="tmp")
                    sss = sin_sb[:, sj * 512 : (sj + 1) * 512]
                    css = cos_sb[:, sj * 512 : (sj + 1) * 512]
                    nc.vector.tensor_mul(tmp[:], rp[:], sss)
                    nc.vector.tensor_mul(t[:], t[:], css)
                    nc.vector.tensor_add(t[:], t[:], tmp[:])
                    store[(h, sj)] = t

                for sj in range(SJ):
                    xt = xts[sj]
                    if sj == 0:
                        # split first-chunk DMAs per f-tile so matmul ft=0 can
                        # start after ~256KB instead of ~6MB
                        for ft in range(FT):
                            nc.sync.dma_start(
                                wq_sb[:, ft, :], wqT[ft * 128 : (ft + 1) * 128, :]
                            )
                            nc.sync.dma_start(
                                xt[:, ft, :], xT[ft * 128 : (ft + 1) * 128, 0:512]
                            )
                        nc.sync.dma_start(rmat_sb[:], rmat[:])
                        nc.sync.dma_start(bq_sb[:], bqT[:])
                        nc.sync.dma_start(bk_sb[:], bkT[:])
                        nc.sync.dma_start(
                            wk_sb[:], wkT.rearrange("(ft p) d -> p ft d", p=128)
                        )
                        nc.sync.dma_start(cos_sb[:, 0:512], cosT[:, 0:512])
                        nc.sync.dma_start(sin_sb[:, 0:512], sinT[:, 0:512])
                        nc.sync.dma_start(
                            wv_sb[:], wvT.rearrange("(ft p) d -> p ft d", p=128)
                        )
                        nc.sync.dma_start(bv_sb[:], bv[:])
                        # prefetch next x chunk ahead of the bulky constants
                        nc.sync.dma_start(
                            xts[1][:],
                            xT[:, 512:1024].rearrange("(ft p) s -> p ft s", p=128),
                        )
                        nc.sync.dma_start(cos_sb[:, 512:], cosT[:, 512:])
                        nc.sync.dma_start(sin_sb[:, 512:], sinT[:, 512:])
                    elif sj + 1 < SJ:
                        nc.sync.dma_start(
                            xts[sj + 1][:],
                            xT[:, (sj + 1) * 512 : (sj + 2) * 512].rearrange(
                                "(ft p) s -> p ft s", p=128
                            ),
                        )

                    qp = [pa.tile([128, 512], F32, tag="ps", name="ps") for _ in range(HPG)]
                    for ft in range(FT):
                        for h in range(HPG):
                            nc.tensor.matmul(
                                qp[h][:],
                                wq_sb[:, ft, h * 128 : (h + 1) * 128],
                                xt[:, ft, :],
                                start=(ft == 0),
                                stop=(ft == FT - 1),
                            )
                    kp = [pa.tile([128, 512], F32, tag="ps", name="ps") for _ in range(HPG)]
                    for ft in range(FT):
                        for h in range(HPG):
                            nc.tensor.matmul(
                                kp[h][:],
                                wk_sb[:, ft, h * 128 : (h + 1) * 128],
                                xt[:, ft, :],
                                start=(ft == 0),
                                stop=(ft == FT - 1),
                            )
                    # Q evacuations (ACT+DVE) run under the K matmuls; the
                    # rope matmuls slot in right after K on the PE queue.
                    for h in range(HPG):
                        evac_qk("q", h, qp[h], bq_sb, QT, sj)
                    for h in range(HPG):
                        evac_qk("k", h, kp[h], bk_sb, KT, sj)
                    # V matmuls reuse xt; they depend only on wv
                    for ss in range(4):
                        vp = pa.tile([128, 512], F32, tag="ps", name="vp")
                        for ft in range(FT):
                            nc.tensor.matmul(
                                vp[:],
                                xt[:, ft, ss * 128 : (ss + 1) * 128],
                                wv_sb[:, ft, :],
                                start=(ft == 0),
                                stop=(ft == FT - 1),
                            )
                        v = vt_pool.tile([128, DLOC], BF16, tag="v", name="v")
                        nc.vector.tensor_add(v[:], vp[:], bv_sb[:])
                        VT[4 * sj + ss] = v
                    if sj == 1:
                        # phase-B constants ride behind xt2 on the FIFO
                        if mode == "causal":
                            nc.sync.dma_start(
                                dm_sb[:], dmask.rearrange("a p t -> p a t")
                            )
                        nc.sync.dma_start(
                            wo_sb[:], woT.rearrange("(dt p) o -> p dt o", p=128)
                        )

            # ============ Phase B: attention + output projection ============
            with (
                tc.tile_pool(name="pst", bufs=2, space="PSUM") as psum_st,
                tc.tile_pool(name="ppv", bufs=2, space="PSUM") as psum_pv,
                tc.tile_pool(name="pms", bufs=2, space="PSUM") as psum_ms,
                tc.tile_pool(name="ex", bufs=3) as exp_pool,
                tc.tile_pool(name="ea", bufs=2) as eacc_pool,
                tc.tile_pool(name="ot", bufs=2 * HPG) as ot_pool,
                tc.tile_pool(name="rc", bufs=4) as rc_pool,
                tc.tile_pool(name="ysb", bufs=4) as y_pool,
                tc.tile_pool(name="fb", bufs=3) as fb_pool,
            ):
                pending = []  # single-matmul out-proj closures (PE filler)

                def pop_filler(n=1):
                    for _ in range(min(n, len(pending))):
                        pending.pop(0)()

                for qj in range(SJ):
                    OT = {}
                    PV = {}
                    RCH = {}

                    def _normalize(i):
                        rcb_ps = psum_ms.tile([128, 512], F32, tag="ms", name="rcb_ps")
                        nc.tensor.matmul(
                            rcb_ps[:], onesr_sb[:], RCH[i][:], start=True, stop=True
                        )
                        rcb = rc_pool.tile([128, 512], F16, tag="rcb", name="rcb")
                        nc.vector.tensor_copy(rcb[:], rcb_ps[:])
                        ot = ot_pool.tile([128, 512], BF16, tag="ot", name="ot")
                        nc.vector.tensor_mul(ot[:], PV[i][:], rcb[:])
                        OT[i] = ot

                    kmax = 4 * qj + 4 if mode == "causal" else KT128
                    P = kmax // 2
                    for h in range(HPG):
                        pv = psum_pv.tile([128, 512], F32, tag="pv", name="pv")
                        eacc = eacc_pool.tile([128, 512], F16, tag="ea", name="ea")

                        def do_pv(e2p, p_, first, last):
                            for half in (0, 1):
                                kj = 2 * p_ + half
                                nc.tensor.matmul(
                                    pv[:],
                                    VT[kj][:, h * 128 : (h + 1) * 128],
                                    e2p[:, half * 512 : (half + 1) * 512],
                                    start=(first and half == 0),
                                    stop=(last and half == 1),
                                )
                            if first:
                                nc.vector.tensor_add(
                                    eacc[:], e2p[:, 0:512], e2p[:, 512:1024]
                                )
                            else:
                                nc.vector.tensor_add(eacc[:], eacc[:], e2p[:, 0:512])
                                nc.vector.tensor_add(eacc[:], eacc[:], e2p[:, 512:1024])

                        prev = None
                        for p in range(P):
                            st = psum_st.tile([128, 1024], F32, tag="st", name="st")
                            for half in (0, 1):
                                kj = 2 * p + half
                                nc.tensor.matmul(
                                    st[:, half * 512 : (half + 1) * 512],
                                    KT[(h, kj // 4)][:, (kj % 4) * 128 : (kj % 4 + 1) * 128],
                                    QT[(h, qj)][:],
                                    start=True,
                                    stop=True,
                                )
                            if mode == "causal" and p >= 2 * qj:
                                variant = p - 2 * qj  # 0 or 1
                                nc.vector.tensor_add(
                                    st[:], st[:], dm_sb[:, variant, :]
                                )
                            elif mode == "bias":
                                fb = fb_pool.tile([128, 1024], F32, tag="fb", name="fb")
                                for half in (0, 1):
                                    kj = 2 * p + half
                                    nc.sync.dma_start(
                                        fb[:, half * 512 : (half + 1) * 512],
                                        fbias[
                                            kj * 128 : (kj + 1) * 128,
                                            qj * 512 : (qj + 1) * 512,
                                        ],
                                    )
                                nc.vector.tensor_add(st[:], st[:], fb[:])
                            e2 = exp_pool.tile([128, 1024], BF16, tag="e", name="e")
                            nc.scalar.activation(
                                e2[:], st[:], mybir.ActivationFunctionType.Exp
                            )
                            if prev is not None:
                                do_pv(prev, p - 1, p - 1 == 0, False)
                                pop_filler(1)
                            prev = e2
                        do_pv(prev, P - 1, P == 1, True)
                        pop_filler(1)

                        dnp = psum_ms.tile([1, 512], F32, tag="ms", name="dnp")
                        nc.tensor.matmul(
                            dnp[:], ones_sb[:], eacc[:], start=True, stop=True
                        )
                        rcf = rc_pool.tile([1, 512], F32, tag="rcf", name="rcf")
                        nc.vector.reciprocal_approx_fast(rcf[:], dnp[:])
                        rch = rc_pool.tile([1, 512], F16, tag="rch", name="rch")
                        nc.vector.tensor_copy(rch[:], rcf[:])
                        PV[h] = pv
                        RCH[h] = rch
                        if h > 0:
                            _normalize(h - 1)
                        if h == HPG - 1:
                            _normalize(h)

                    # queue this qj's output projection as PE filler for the
                    # next qj's attention
                    OTs = [OT[dt] for dt in range(HPG)]
                    r_qj = qj

                    def mk(ss, oj, OTl, qjl):
                        state = {}

                        def go():
                            yp = psum_ms.tile([128, 512], F32, tag="ms", name="yp")
                            for dt in range(HPG):
                                nc.tensor.matmul(
                                    yp[:],
                                    OTl[dt][:, ss * 128 : (ss + 1) * 128],
                                    wo_sb[:, dt, oj * 512 : (oj + 1) * 512],
                                    start=(dt == 0),
                                    stop=(dt == HPG - 1),
                                )
                            ysb = y_pool.tile([128, 512], BF16, tag="y", name="y")
                            nc.vector.tensor_copy(ysb[:], yp[:])
                            r0 = qjl * 512 + ss * 128
                            nc.sync.dma_start(
                                y[r0 : r0 + 128, oj * 512 : (oj + 1) * 512], ysb[:]
                            )

                        return go

                    for oj in range(4):
                        for ss in range(4):
                            pending.append(mk(ss, oj, OTs, r_qj))
                pop_filler(len(pending))
    nc.compile()
    return nc


_PROGRAM_CACHE = {}


def _get_program(mode):
    if mode not in _PROGRAM_CACHE:
        _PROGRAM_CACHE[mode] = build_program(mode)
    return _PROGRAM_CACHE[mode]


def _detect_mode(attn_mask):
    m = np.asarray(attn_mask).reshape(S, S)
    if (m == np.tril(np.ones((S, S), m.dtype))).all():
        return "causal"
    if (m != 0).all():
        return "full"
    return "bias"


def _rot_matrix():
    # rot(q)[d'] = -q[d'+1] (d' even), +q[d'-1] (d' odd);  rotT = R^T @ qT with
    # lhsT[d, d'] convention of nc.tensor.matmul.
    r = np.zeros((HD, HD), np.float32)
    for dp in range(HD):
        if dp % 2 == 0:
            r[dp + 1, dp] = -1.0
        else:
            r[dp - 1, dp] = 1.0
    return r


def _diag_mask2():
    # [2 variants, 128 rows(k), 1024 cols]: variant v half hf covers k-tile
    # a = 2v+hf of the diagonal group; allowed iff col >= 128*a + row.
    out = np.zeros((2, 128, 1024), np.float32)
    r = np.arange(128)[:, None]
    c = np.arange(512)[None, :]
    for v in range(2):
        for hf in range(2):
            a = 2 * v + hf
            out[v, :, hf * 512 : (hf + 1) * 512] = np.where(
                c >= 128 * a + r, 0.0, NEG
            )
    return out


def _bf16(a):
    return np.ascontiguousarray(a).astype(NPBF16)


def kernel(**inputs) -> np.ndarray:
    from concourse.bass_utils import run_bass_kernel_spmd

    x = np.asarray(inputs["x"], np.float32)
    fcos = np.asarray(inputs["fcos"], np.float32)
    fsin = np.asarray(inputs["fsin"], np.float32)
    Wq, bq = np.asarray(inputs["Wq"], np.float32), np.asarray(inputs["bq"], np.float32)
    Wk, bk = np.asarray(inputs["Wk"], np.float32), np.asarray(inputs["bk"], np.float32)
    Wv, bv = np.asarray(inputs["Wv"], np.float32), np.asarray(inputs["bv"], np.float32)
    Wo, bo = np.asarray(inputs["Wo"], np.float32), np.asarray(inputs["bo"], np.float32)
    attn_mask = inputs["attn_mask"]

    mode = _detect_mode(attn_mask)
    nc = _get_program(mode)

    sc = 1.0 / math.sqrt(HD)
    shared = {
        "cosT": np.ascontiguousarray(fcos.T),
        "sinT": np.ascontiguousarray(fsin.T),
        "rmat": _rot_matrix().astype(NPBF16),
    }
    if mode == "causal":
        shared["dmask"] = _diag_mask2()
    elif mode == "bias":
        m = np.asarray(attn_mask).reshape(S, S)
        shared["fbias"] = np.ascontiguousarray(
            np.where(m.T == 0, NEG, 0.0).astype(np.float32)
        )

    in_maps = []
    for c in range(NCORES):
        b, hg = divmod(c, HG)
        rows = slice(DLOC * hg, DLOC * (hg + 1))
        in_maps.append(
            {
                "xT": _bf16(x[b].T),
                "wqT": _bf16((Wq[rows] * sc).T),
                "wkT": _bf16(Wk[rows].T),
                "wvT": _bf16(Wv[rows].T),
                "woT": _bf16(Wo[:, rows].T),
                "bqT": np.ascontiguousarray((bq[rows] * sc).reshape(HPG, 128).T),
                "bkT": np.ascontiguousarray(bk[rows].reshape(HPG, 128).T),
                "bv": np.ascontiguousarray(
                    np.broadcast_to(bv[rows].reshape(1, DLOC), (128, DLOC))
                ).astype(np.float32),
                **shared,
            }
        )

    trace = bool(int(os.environ.get("KERNEL_TRACE", "0")))
    res = run_bass_kernel_spmd(nc, in_maps, list(range(NCORES)), trace=trace)
    if trace and res.exec_time_ns is not None:
        print(f"HW exec time: {res.exec_time_ns} ns")
        globals()["LAST_EXEC_NS"] = res.exec_time_ns
        globals()["LAST_RESULTS"] = res

    out = np.zeros((B, S, H), np.float32)
    for c in range(NCORES):
        out[c // HG] += res.results[c]["y"].astype(np.float32)
    out += bo
    return out
